# revision 9
# baseline (speedup 1.0000x reference)
"""Trainium2 Bass kernel for nn_MPCActor: MLP (256->512->512->32, relu/relu/
sigmoid) followed by 100 SGD steps on u (closed form: u <- a*u + b per element
with a = 1-2*lr*q has exact solution u_N = A*(u0 + p/(2q)) - p/(2q), A = a^N).

Data parallel over 8 NeuronCores: batch 32768 -> 4096 rows per core, MLP
weights replicated. All matmuls run in fp8 (e4m3) with DoubleRow perf mode
(two k-planes per pass, 2x bf16 throughput); accumulation is fp32 in PSUM.
Weights are pre-scaled on host so fp8 operands sit in the normal range:
W1*64 (y1 carries 64x), W2*4 (y2 carries 256x), W3*64 (psum3 = 16384*z3,
folded into the sigmoid's scale). e4m3 max-finite is 240; scaled activations
peak around 150.

obs is transposed + cast to fp8 on host (layout prep, like the weight
slicing), so the kernel has no PE transposes and 4x less obs DMA; the
feature-major activations feed matmuls directly. Only the 8 W3 columns the
u-update reads (q_u = cols 12:16, p_u = cols 28:32) are computed.

Engine split per batch tile: PE does 14 DoubleRow matmuls; the 8 PSUM relu
drains rotate over ACT / DVE / GPSIMD; ACT also does the sigmoid and the
a^100 squaring chain; the tiny per-tile closed-form ops spread over DVE/Pool.
"""

import numpy as np
import ml_dtypes

import concourse.bass as bass
import concourse.mybir as mybir
import concourse.tile as tile
from concourse import bacc, masks
from concourse.bass_utils import run_bass_kernel_spmd

NCORES = 8
BATCH = 32768
BPC = BATCH // NCORES  # 4096 rows per core
OBS = 256
HID = 512
NQP = 8  # q_u (4) + p_u (4) columns of W3 that matter
NQPP = 32  # padded: dual-fp8 LDWEIGHTS needs >=16 cols; 32 avoids narrow-psum slowdown
BT = 512  # batch tile (matmul moving free dim)
NT = BPC // BT  # 8 batch tiles per core
LR = 0.01
F32 = mybir.dt.float32
BF16 = mybir.dt.bfloat16
F8 = mybir.dt.float8e4
F8NP = mybir.dt.np(F8)  # ml_dtypes.float8_e4m3 (max finite 240)
DR = mybir.MatmulPerfMode.DoubleRow

# fp8 scale plan: y1 tilde = S1*y1, y2 tilde = S2*y2 (e4m3 max finite = 240;
# scaled activations peak ~120, giving 2x saturation margin)
S1 = 64.0
S2 = 128.0
W2S = S2 / S1  # 2.0
W3S = 64.0
Z3S = S2 * W3S  # psum3 = 8192 * (z3 - b3)

_CACHE = {}


def _build_nc():
    nc = bacc.Bacc(
        trn_type="TRN2", target_bir_lowering=False, debug=False, num_devices=NCORES
    )
    # obsT: [128, 2, BPC] fp8, element [p, kc, b] = obs[b, kc*128+p]
    obsT = nc.declare_dram_parameter("obsT", [128, 2, BPC], F8, isOutput=False).ap()
    u0 = nc.declare_dram_parameter("u0", [BPC, 4], F32, isOutput=False).ap()
    w1 = nc.declare_dram_parameter("w1", [128, 2, HID], F8, isOutput=False).ap()
    w2 = nc.declare_dram_parameter("w2", [128, 4, HID], F8, isOutput=False).ap()
    w3 = nc.declare_dram_parameter("w3", [128, 4, NQPP], F8, isOutput=False).ap()
    b1 = nc.declare_dram_parameter("b1", [128, 4], F32, isOutput=False).ap()
    b2 = nc.declare_dram_parameter("b2", [128, 4], F32, isOutput=False).ap()
    b3 = nc.declare_dram_parameter("b3", [NQP, 1], F32, isOutput=False).ap()
    uo = nc.declare_dram_parameter("uo", [BPC, 4], F32, isOutput=True).ap()

    AF = mybir.ActivationFunctionType
    ALU = mybir.AluOpType

    with tile.TileContext(nc) as tc:
        from contextlib import ExitStack

        with ExitStack() as ctx:
            singles = ctx.enter_context(tc.tile_pool(name="singles", bufs=1))
            p_y1 = ctx.enter_context(tc.tile_pool(name="y1", bufs=2))
            p_y2 = ctx.enter_context(tc.tile_pool(name="y2", bufs=2))
            p_qp = ctx.enter_context(tc.tile_pool(name="qp", bufs=2))
            p_cf = ctx.enter_context(tc.tile_pool(name="cf", bufs=2))
            # PSUM budget 8 banks: y1 3 + y2 3 + z3 1 + qpt 1
            pp_y1 = ctx.enter_context(tc.tile_pool(name="ppy1", bufs=3, space="PSUM"))
            pp_y2 = ctx.enter_context(tc.tile_pool(name="ppy2", bufs=3, space="PSUM"))
            pp_z3 = ctx.enter_context(tc.tile_pool(name="ppz3", bufs=1, space="PSUM"))
            pp_qpt = ctx.enter_context(tc.tile_pool(name="ppqpt", bufs=1, space="PSUM"))

            # ---- one-time loads: fp8 obsT + weights, f32 biases, identity.
            # w1 first (gates the first matmul), then obs in chunks so tile 0
            # compute starts after ~1/4 of the obs transfer ----
            w1s = singles.tile([128, 2, HID], F8)
            nc.sync.dma_start(out=w1s, in_=w1)
            NCHUNK = 4
            CW = BPC // NCHUNK
            obsC = []
            for ci in range(NCHUNK):
                oc = singles.tile([128, 2, CW], F8, name=f"obsC{ci}", tag=f"obsC{ci}")
                obsC.append(oc)
            nc.sync.dma_start(out=obsC[0], in_=obsT[:, :, 0:CW])
            w2s = singles.tile([128, 4, HID], F8)
            nc.sync.dma_start(out=w2s, in_=w2)
            w3s = singles.tile([128, 4, NQPP], F8)
            nc.sync.dma_start(out=w3s, in_=w3)
            for ci in range(1, NCHUNK):
                nc.sync.dma_start(out=obsC[ci], in_=obsT[:, :, ci * CW : (ci + 1) * CW])
            b1s = singles.tile([128, 4], F32)
            nc.sync.dma_start(out=b1s, in_=b1)
            b2s = singles.tile([128, 4], F32)
            nc.sync.dma_start(out=b2s, in_=b2)
            b3s = singles.tile([NQP, 1], F32)
            nc.sync.dma_start(out=b3s, in_=b3)
            id8 = singles.tile([NQP, NQP], BF16)
            masks.make_identity(nc, id8[:])

            u0_t = u0.rearrange("(t c p) j -> p t c j", p=128, c=4)
            uo_t = uo.rearrange("(t c p) j -> p t c j", p=128, c=4)
            u0_all = singles.tile([128, NT, 4, 4], F32)
            nc.sync.dma_start(out=u0_all, in_=u0_t)
            qp_all = singles.tile([128, NT, 4, NQP], F32)

            # engine rotation for the 8 relu drains of each tile: ACT 4 / DVE 4
            # (GPSIMD cannot read PSUM on TRN2; it gets the SBUF-only
            # closed-form chain instead)
            def drain(dst, src, bias_ap, slot):
                if slot % 2 == 0:
                    nc.scalar.activation(
                        out=dst, in_=src, func=AF.Relu, bias=bias_ap, scale=1.0
                    )
                else:
                    nc.vector.tensor_scalar(dst, src, bias_ap, 0.0, ALU.add, ALU.max)

            TPC = CW // BT  # tiles per obs chunk
            for it in range(NT):
                oc = obsC[it // TPC]
                base = (it % TPC) * BT
                rhs1 = oc[:, :, base : base + BT]

                # layer 1: one DoubleRow matmul per 128-wide m chunk
                y1 = p_y1.tile([128, 4, BT], F8, tag="y1")
                for m in range(4):
                    ps = pp_y1.tile([128, BT], F32, tag="psy1")
                    nc.tensor.matmul(
                        ps,
                        w1s[:, :, m * 128 : (m + 1) * 128],
                        rhs1,
                        start=True,
                        stop=True,
                        perf_mode=DR,
                    )
                    drain(y1[:, m, :], ps, b1s[:, m : m + 1], m)

                # layer 2: two DoubleRow matmuls (k pairs) per m chunk
                y2 = p_y2.tile([128, 4, BT], F8, tag="y2")
                for m in range(4):
                    ps = pp_y2.tile([128, BT], F32, tag="psy2")
                    for i in range(2):
                        nc.tensor.matmul(
                            ps,
                            w2s[:, 2 * i : 2 * i + 2, m * 128 : (m + 1) * 128],
                            y1[:, 2 * i : 2 * i + 2, :],
                            start=(i == 0),
                            stop=(i == 1),
                            perf_mode=DR,
                        )
                    drain(y2[:, m, :], ps, b2s[:, m : m + 1], m + 4)

                # layer 3: only the 8 useful output columns; sigmoid on ACT
                ps3 = pp_z3.tile([NQPP, BT], F32, tag="z3")
                for i in range(2):
                    nc.tensor.matmul(
                        ps3,
                        w3s[:, 2 * i : 2 * i + 2, :],
                        y2[:, 2 * i : 2 * i + 2, :],
                        start=(i == 0),
                        stop=(i == 1),
                        perf_mode=DR,
                    )
                qpT = p_qp.tile([NQP, BT], BF16, tag="qpT")
                nc.scalar.activation(
                    out=qpT,
                    in_=ps3[0:NQP, :],
                    func=AF.Sigmoid,
                    bias=b3s[:, 0:1],
                    scale=1.0 / Z3S,
                )

                # transpose to batch-major [128, 4, 8] (out free = 8 -> cheap)
                psq = pp_qpt.tile([128, 4, NQP], BF16, tag="qpt")
                for c in range(4):
                    nc.tensor.transpose(
                        psq[:, c, :], qpT[:, c * 128 : (c + 1) * 128], id8[:]
                    )

                # stash batch-major qp for the batched closed form at the end
                nc.vector.tensor_copy(out=qp_all[:, it], in_=psq)

            # ---- batched closed-form 100-step update over all tiles ----
            # u_N = A*(u0 + w) - w, w = p/(2q), A = (1 - 2*lr*q)^100
            q = qp_all[:, :, :, 0:4]
            p4 = qp_all[:, :, :, 4:8]
            SH = [128, NT, 4, 4]
            a = p_cf.tile(SH, F32, tag="a")  # a = 1 - 2*lr*q
            nc.scalar.activation(out=a, in_=q, func=AF.Copy, bias=1.0, scale=-2.0 * LR)
            rq = p_cf.tile(SH, F32, tag="rq")
            nc.vector.reciprocal(rq, q)
            w = p_cf.tile(SH, F32, tag="w")
            nc.vector.scalar_tensor_tensor(
                out=w, in0=p4, scalar=0.5, in1=rq, op0=ALU.mult, op1=ALU.mult
            )
            a2 = p_cf.tile(SH, F32, tag="a2")
            nc.scalar.square(a2, a)
            a4 = p_cf.tile(SH, F32, tag="a4")
            nc.scalar.square(a4, a2)
            a8 = p_cf.tile(SH, F32, tag="a8")
            nc.scalar.square(a8, a4)
            a16 = p_cf.tile(SH, F32, tag="a16")
            nc.scalar.square(a16, a8)
            a32 = p_cf.tile(SH, F32, tag="a32")
            nc.scalar.square(a32, a16)
            a64 = p_cf.tile(SH, F32, tag="a64")
            nc.scalar.square(a64, a32)
            a96 = p_cf.tile(SH, F32, tag="a96")
            nc.gpsimd.tensor_mul(a96, a64, a32)
            A = p_cf.tile(SH, F32, tag="A")
            nc.vector.tensor_mul(A, a96, a4)
            s_ = p_cf.tile(SH, F32, tag="s_")
            nc.gpsimd.tensor_add(s_, u0_all, w)
            us = p_cf.tile(SH, F32, tag="us")
            nc.vector.tensor_mul(us, A, s_)
            uob = p_cf.tile(SH, F32, tag="uob")
            nc.gpsimd.tensor_sub(uob, us, w)
            nc.sync.dma_start(out=uo_t, in_=uob)
    nc.finalize()
    return nc


def _get_nc():
    if "nc" not in _CACHE:
        _CACHE["nc"] = _build_nc()
    return _CACHE["nc"]


def kernel(obs, x_init, u_init, W1, b1, W2, b2, W3, b3):
    obs = np.asarray(obs, dtype=np.float32)
    u_init = np.ascontiguousarray(np.asarray(u_init, dtype=np.float32))
    W1 = np.asarray(W1, dtype=np.float32)
    W2 = np.asarray(W2, dtype=np.float32)
    W3 = np.asarray(W3, dtype=np.float32)
    b1 = np.asarray(b1, dtype=np.float32)
    b2 = np.asarray(b2, dtype=np.float32)
    b3 = np.asarray(b3, dtype=np.float32)

    # weights to fp8 with scaling; [k, m] -> [128, kc, m] (k = kc*128 + p)
    w1c = np.ascontiguousarray(
        (S1 * W1).reshape(2, 128, HID).transpose(1, 0, 2).astype(F8NP)
    )
    w2c = np.ascontiguousarray(
        (W2S * W2).reshape(4, 128, HID).transpose(1, 0, 2).astype(F8NP)
    )
    # only columns 12:16 (q_u) and 28:32 (p_u) of the MLP head are used
    w3u = np.concatenate([W3[:, 12:16], W3[:, 28:32]], axis=1)
    w3p = np.concatenate([W3S * w3u, np.zeros((HID, NQPP - NQP), np.float32)], 1)
    w3c = np.ascontiguousarray(
        w3p.reshape(4, 128, NQPP).transpose(1, 0, 2).astype(F8NP)
    )
    b1p = np.ascontiguousarray((S1 * b1).reshape(4, 128).T)
    b2p = np.ascontiguousarray((S2 * b2).reshape(4, 128).T)
    b3u = np.ascontiguousarray(np.concatenate([b3[12:16], b3[28:32]])[:, None])

    nc = _get_nc()
    in_maps = []
    for i in range(NCORES):
        obs_i = obs[i * BPC : (i + 1) * BPC]  # [BPC, 256]
        # [p, kc, b] = obs[b, kc*128+p]
        obsT_i = np.ascontiguousarray(
            obs_i.T.reshape(2, 128, BPC).transpose(1, 0, 2).astype(F8NP)
        )
        in_maps.append(
            {
                "obsT": obsT_i,
                "u0": u_init[i * BPC : (i + 1) * BPC],
                "w1": w1c,
                "w2": w2c,
                "w3": w3c,
                "b1": b1p,
                "b2": b2p,
                "b3": b3u,
            }
        )
    import os

    kw = {}
    if os.environ.get("BASSK_TRACE"):
        kw = {"trace": True, "tmpdir": os.environ.get("BASSK_TRACE_DIR") or None}
    res = run_bass_kernel_spmd(nc, in_maps, list(range(NCORES)), **kw)
    _CACHE["last_result"] = res
    out = np.concatenate([res.results[i]["uo"] for i in range(NCORES)], axis=0)
    return out.astype(np.float32)


# revision 12
# speedup vs baseline: 1.1948x; 1.1948x over previous
"""Trainium2 Bass kernel for nn_MPCActor: MLP (256->512->512->32, relu/relu/
sigmoid) followed by 100 SGD steps on u (closed form: u <- a*u + b per element
with a = 1-2*lr*q has exact solution u_N = A*(u0 + p/(2q)) - p/(2q), A = a^N).

Data parallel over 8 NeuronCores: batch 32768 -> 4096 rows per core, MLP
weights replicated. All matmuls run in fp8 (e4m3) with DoubleRow perf mode
(two k-planes per pass, 2x bf16 throughput); accumulation is fp32 in PSUM.
Weights are pre-scaled on host so fp8 operands sit in the normal range:
W1*64 (y1 carries 64x), W2*4 (y2 carries 256x), W3*64 (psum3 = 16384*z3,
folded into the sigmoid's scale). e4m3 max-finite is 240; scaled activations
peak around 150.

obs is transposed + cast to fp8 on host (layout prep, like the weight
slicing), so the kernel has no PE transposes and 4x less obs DMA; the
feature-major activations feed matmuls directly. Only the 8 W3 columns the
u-update reads (q_u = cols 12:16, p_u = cols 28:32) are computed.

Engine split per batch tile: PE does 14 DoubleRow matmuls; the 8 PSUM relu
drains rotate over ACT / DVE / GPSIMD; ACT also does the sigmoid and the
a^100 squaring chain; the tiny per-tile closed-form ops spread over DVE/Pool.
"""

import numpy as np
import ml_dtypes

import concourse.bass as bass
import concourse.mybir as mybir
import concourse.tile as tile
from concourse import bacc, masks
from concourse.bass_utils import run_bass_kernel_spmd

NCORES = 8
BATCH = 32768
BPC = BATCH // NCORES  # 4096 rows per core
OBS = 256
HID = 512
NQP = 8  # q_u (4) + p_u (4) columns of W3 that matter
NQPP = 32  # padded: dual-fp8 LDWEIGHTS needs >=16 cols; 32 avoids narrow-psum slowdown
BT = 512  # batch tile (matmul moving free dim)
NT = BPC // BT  # 8 batch tiles per core
LR = 0.01
F32 = mybir.dt.float32
BF16 = mybir.dt.bfloat16
F8 = mybir.dt.float8e4
F8NP = mybir.dt.np(F8)  # ml_dtypes.float8_e4m3 (max finite 240)
DR = mybir.MatmulPerfMode.DoubleRow

# fp8 scale plan: y1 tilde = S1*y1, y2 tilde = S2*y2 (e4m3 max finite = 240;
# scaled activations peak ~120, giving 2x saturation margin)
S1 = 64.0
S2 = 128.0
W2S = S2 / S1  # 2.0
W3S = 64.0
Z3S = S2 * W3S  # psum3 = 8192 * (z3 - b3)

_CACHE = {}


def _build_nc():
    nc = bacc.Bacc(
        trn_type="TRN2", target_bir_lowering=False, debug=False, num_devices=NCORES
    )
    # obsT: [128, 2, BPC] fp8, element [p, kc, b] = obs[b, kc*128+p]
    obsT = nc.declare_dram_parameter("obsT", [128, 2, BPC], F8, isOutput=False).ap()
    u0 = nc.declare_dram_parameter("u0", [128, NT, 4, 4], F32, isOutput=False).ap()
    w1 = nc.declare_dram_parameter("w1", [128, 2, HID], F8, isOutput=False).ap()
    w2 = nc.declare_dram_parameter("w2", [128, 4, HID], F8, isOutput=False).ap()
    w3 = nc.declare_dram_parameter("w3", [128, 4, NQPP], F8, isOutput=False).ap()
    b1 = nc.declare_dram_parameter("b1", [128, 4], F32, isOutput=False).ap()
    b2 = nc.declare_dram_parameter("b2", [128, 4], F32, isOutput=False).ap()
    b3 = nc.declare_dram_parameter("b3", [NQP, 1], F32, isOutput=False).ap()
    uo = nc.declare_dram_parameter("uo", [128, NT, 4, 4], F32, isOutput=True).ap()

    AF = mybir.ActivationFunctionType
    ALU = mybir.AluOpType

    with tile.TileContext(nc) as tc:
        from contextlib import ExitStack

        with ExitStack() as ctx:
            singles = ctx.enter_context(tc.tile_pool(name="singles", bufs=1))
            p_y1 = ctx.enter_context(tc.tile_pool(name="y1", bufs=2))
            p_y2 = ctx.enter_context(tc.tile_pool(name="y2", bufs=2))
            p_qp = ctx.enter_context(tc.tile_pool(name="qp", bufs=2))
            p_cf = ctx.enter_context(tc.tile_pool(name="cf", bufs=2))
            # PSUM budget 8 banks: y1 3 + y2 3 + z3 1 + qpt 1
            pp_y1 = ctx.enter_context(tc.tile_pool(name="ppy1", bufs=3, space="PSUM"))
            pp_y2 = ctx.enter_context(tc.tile_pool(name="ppy2", bufs=3, space="PSUM"))
            pp_z3 = ctx.enter_context(tc.tile_pool(name="ppz3", bufs=1, space="PSUM"))
            pp_qpt = ctx.enter_context(tc.tile_pool(name="ppqpt", bufs=1, space="PSUM"))

            # ---- one-time loads: fp8 obsT + weights, f32 biases, identity.
            # w1 first (gates the first matmul), then obs in chunks so tile 0
            # compute starts after ~1/4 of the obs transfer ----
            w1s = singles.tile([128, 2, HID], F8)
            nc.sync.dma_start(out=w1s, in_=w1)
            NCHUNK = 4
            CW = BPC // NCHUNK
            obsC = []
            for ci in range(NCHUNK):
                oc = singles.tile([128, 2, CW], F8, name=f"obsC{ci}", tag=f"obsC{ci}")
                obsC.append(oc)
            nc.gpsimd.dma_start(out=obsC[0], in_=obsT[:, :, 0:CW])
            w2s = singles.tile([128, 4, HID], F8)
            nc.sync.dma_start(out=w2s, in_=w2)
            w3s = singles.tile([128, 4, NQPP], F8)
            nc.sync.dma_start(out=w3s, in_=w3)
            for ci in range(1, NCHUNK):
                nc.gpsimd.dma_start(
                    out=obsC[ci], in_=obsT[:, :, ci * CW : (ci + 1) * CW]
                )
            b1s = singles.tile([128, 4], F32)
            nc.sync.dma_start(out=b1s, in_=b1)
            b2s = singles.tile([128, 4], F32)
            nc.sync.dma_start(out=b2s, in_=b2)
            b3s = singles.tile([NQP, 1], F32)
            nc.sync.dma_start(out=b3s, in_=b3)
            id8 = singles.tile([NQP, NQP], BF16)
            masks.make_identity(nc, id8[:])

            u0_all = singles.tile([128, NT, 4, 4], F32)
            nc.sync.dma_start(out=u0_all, in_=u0)
            qp_all = singles.tile([128, NT, 4, NQP], F32)

            # engine rotation for the 8 relu drains of each tile: ACT 4 / DVE 4
            # (GPSIMD cannot read PSUM on TRN2; it gets the SBUF-only
            # closed-form chain instead)
            def drain(dst, src, bias_ap, slot):
                if slot % 2 == 0:
                    nc.scalar.activation(
                        out=dst, in_=src, func=AF.Relu, bias=bias_ap, scale=1.0
                    )
                else:
                    nc.vector.tensor_scalar(dst, src, bias_ap, 0.0, ALU.add, ALU.max)

            # ---- batched closed-form 100-step update over a tile range ----
            # u_N = A*(u0 + w) - w, w = p/(2q), A = (1 - 2*lr*q)^100
            def closed_form(t0, t1):
                q = qp_all[:, t0:t1, :, 0:4]
                p4 = qp_all[:, t0:t1, :, 4:8]
                u0h = u0_all[:, t0:t1]
                SH = [128, t1 - t0, 4, 4]
                g = f"cf{t0}"
                a = p_cf.tile(SH, F32, tag=f"a{g}")  # a = 1 - 2*lr*q
                nc.scalar.activation(
                    out=a, in_=q, func=AF.Copy, bias=1.0, scale=-2.0 * LR
                )
                rq = p_cf.tile(SH, F32, tag=f"rq{g}")
                nc.vector.reciprocal(rq, q)
                w = p_cf.tile(SH, F32, tag=f"w{g}")
                nc.vector.scalar_tensor_tensor(
                    out=w, in0=p4, scalar=0.5, in1=rq, op0=ALU.mult, op1=ALU.mult
                )
                a2 = p_cf.tile(SH, F32, tag=f"a2{g}")
                nc.scalar.square(a2, a)
                a4 = p_cf.tile(SH, F32, tag=f"a4{g}")
                nc.scalar.square(a4, a2)
                a8 = p_cf.tile(SH, F32, tag=f"a8{g}")
                nc.scalar.square(a8, a4)
                a16 = p_cf.tile(SH, F32, tag=f"a16{g}")
                nc.scalar.square(a16, a8)
                a32 = p_cf.tile(SH, F32, tag=f"a32{g}")
                nc.scalar.square(a32, a16)
                a64 = p_cf.tile(SH, F32, tag=f"a64{g}")
                nc.scalar.square(a64, a32)
                a96 = p_cf.tile(SH, F32, tag=f"a96{g}")
                nc.gpsimd.tensor_mul(a96, a64, a32)
                A = p_cf.tile(SH, F32, tag=f"A{g}")
                nc.vector.tensor_mul(A, a96, a4)
                s_ = p_cf.tile(SH, F32, tag=f"s{g}")
                nc.gpsimd.tensor_add(s_, u0h, w)
                us = p_cf.tile(SH, F32, tag=f"us{g}")
                nc.vector.tensor_mul(us, A, s_)
                uob = p_cf.tile(SH, F32, tag=f"uob{g}")
                nc.gpsimd.tensor_sub(uob, us, w)
                nc.sync.dma_start(out=uo[:, t0:t1], in_=uob)

            TPC = CW // BT  # tiles per obs chunk
            for it in range(NT):
                oc = obsC[it // TPC]
                base = (it % TPC) * BT
                rhs1 = oc[:, :, base : base + BT]

                # layer 1: one DoubleRow matmul per 128-wide m chunk
                y1 = p_y1.tile([128, 4, BT], F8, tag="y1")
                for m in range(4):
                    ps = pp_y1.tile([128, BT], F32, tag="psy1")
                    nc.tensor.matmul(
                        ps,
                        w1s[:, :, m * 128 : (m + 1) * 128],
                        rhs1,
                        start=True,
                        stop=True,
                        perf_mode=DR,
                    )
                    drain(y1[:, m, :], ps, b1s[:, m : m + 1], m)

                # layer 2: two DoubleRow matmuls (k pairs) per m chunk
                y2 = p_y2.tile([128, 4, BT], F8, tag="y2")
                for m in range(4):
                    ps = pp_y2.tile([128, BT], F32, tag="psy2")
                    for i in range(2):
                        nc.tensor.matmul(
                            ps,
                            w2s[:, 2 * i : 2 * i + 2, m * 128 : (m + 1) * 128],
                            y1[:, 2 * i : 2 * i + 2, :],
                            start=(i == 0),
                            stop=(i == 1),
                            perf_mode=DR,
                        )
                    drain(y2[:, m, :], ps, b2s[:, m : m + 1], m + 4)

                # layer 3: only the 8 useful output columns; sigmoid on ACT
                ps3 = pp_z3.tile([NQPP, BT], F32, tag="z3")
                for i in range(2):
                    nc.tensor.matmul(
                        ps3,
                        w3s[:, 2 * i : 2 * i + 2, :],
                        y2[:, 2 * i : 2 * i + 2, :],
                        start=(i == 0),
                        stop=(i == 1),
                        perf_mode=DR,
                    )
                qpT = p_qp.tile([NQP, BT], BF16, tag="qpT")
                nc.scalar.activation(
                    out=qpT,
                    in_=ps3[0:NQP, :],
                    func=AF.Sigmoid,
                    bias=b3s[:, 0:1],
                    scale=1.0 / Z3S,
                )

                # transpose to batch-major [128, 4, 8] (out free = 8 -> cheap)
                psq = pp_qpt.tile([128, 4, NQP], BF16, tag="qpt")
                for c in range(4):
                    nc.tensor.transpose(
                        psq[:, c, :], qpT[:, c * 128 : (c + 1) * 128], id8[:]
                    )

                # stash batch-major qp for the batched closed form
                nc.vector.tensor_copy(out=qp_all[:, it], in_=psq)

                # closed form for a half of the tiles as soon as it's ready:
                # the first half overlaps the second half's matmuls, only the
                # second half's chain is in the tail
                if it == NT // 2 - 1:
                    closed_form(0, NT // 2)
                elif it == NT - 1:
                    closed_form(NT // 2, NT)
    nc.finalize()
    return nc
    nc.finalize()
    return nc


def _get_nc():
    if "nc" not in _CACHE:
        _CACHE["nc"] = _build_nc()
    return _CACHE["nc"]


def kernel(obs, x_init, u_init, W1, b1, W2, b2, W3, b3):
    obs = np.asarray(obs, dtype=np.float32)
    u_init = np.ascontiguousarray(np.asarray(u_init, dtype=np.float32))
    W1 = np.asarray(W1, dtype=np.float32)
    W2 = np.asarray(W2, dtype=np.float32)
    W3 = np.asarray(W3, dtype=np.float32)
    b1 = np.asarray(b1, dtype=np.float32)
    b2 = np.asarray(b2, dtype=np.float32)
    b3 = np.asarray(b3, dtype=np.float32)

    # weights to fp8 with scaling; [k, m] -> [128, kc, m] (k = kc*128 + p)
    w1c = np.ascontiguousarray(
        (S1 * W1).reshape(2, 128, HID).transpose(1, 0, 2).astype(F8NP)
    )
    w2c = np.ascontiguousarray(
        (W2S * W2).reshape(4, 128, HID).transpose(1, 0, 2).astype(F8NP)
    )
    # only columns 12:16 (q_u) and 28:32 (p_u) of the MLP head are used
    w3u = np.concatenate([W3[:, 12:16], W3[:, 28:32]], axis=1)
    w3p = np.concatenate([W3S * w3u, np.zeros((HID, NQPP - NQP), np.float32)], 1)
    w3c = np.ascontiguousarray(
        w3p.reshape(4, 128, NQPP).transpose(1, 0, 2).astype(F8NP)
    )
    b1p = np.ascontiguousarray((S1 * b1).reshape(4, 128).T)
    b2p = np.ascontiguousarray((S2 * b2).reshape(4, 128).T)
    b3u = np.ascontiguousarray(np.concatenate([b3[12:16], b3[28:32]])[:, None])

    nc = _get_nc()
    in_maps = []
    for i in range(NCORES):
        obs_i = obs[i * BPC : (i + 1) * BPC]  # [BPC, 256]
        # [p, kc, b] = obs[b, kc*128+p]
        obsT_i = np.ascontiguousarray(
            obs_i.T.reshape(2, 128, BPC).transpose(1, 0, 2).astype(F8NP)
        )
        u0_i = np.ascontiguousarray(
            u_init[i * BPC : (i + 1) * BPC]
            .reshape(NT, 4, 128, 4)
            .transpose(2, 0, 1, 3)
        )
        in_maps.append(
            {
                "obsT": obsT_i,
                "u0": u0_i,
                "w1": w1c,
                "w2": w2c,
                "w3": w3c,
                "b1": b1p,
                "b2": b2p,
                "b3": b3u,
            }
        )
    import os

    kw = {}
    if os.environ.get("BASSK_TRACE"):
        kw = {"trace": True, "tmpdir": os.environ.get("BASSK_TRACE_DIR") or None}
    res = run_bass_kernel_spmd(nc, in_maps, list(range(NCORES)), **kw)
    _CACHE["last_result"] = res
    # invert the [128, NT, 4, 4] -> [BPC, 4] layout per core
    outs = [
        res.results[i]["uo"].transpose(1, 2, 0, 3).reshape(BPC, 4)
        for i in range(NCORES)
    ]
    return np.concatenate(outs, axis=0).astype(np.float32)
